# revision 31
# baseline (speedup 1.0000x reference)
"""CTLSTM Trainium2 kernel, v3b: per-type tables + h_d only, fused expansion.

The reference re-inits h/c/c_bar to zero every step, so all gate values
depend ONLY on the event type (1001 distinct embedding rows). Output planes
c, c_bar, go, gd are pure per-type gathers: the device returns the four
128x1024 bf16 tables and h_d only; the host's unshard step materializes
those planes with numpy fancy-indexing (same class of data movement as the
host un-permute it already does).

Device (per core, 128-wide H slice, all 16384 tokens host-sorted by type):
  phase 1: gate tables tab[128, 4(delta|cb|go|gd), 1024 types] via matmul
    (weights stationary, embedding columns moving) + ACT activations;
    delta = c - c_bar = (gi - gib)*gz tabulated. gd rows are DMA-transposed
    into gdT[128 types%128, chunk, 128 h] for use as matmul stationary.
  phase 2 (tokens sorted by type; segments are 128-type-chunk-aligned,
    waves pack segments):
    - m = gd * (-dur) comes off the TENSOR engine: one matmul per segment
      with stationary gdT[chunk] and moving a host-built scaled one-hot
      (row = type%128, value = -dur). No gd expansion, no DVE multiply.
    - e = Exp(m) per segment (ACT, PSUM->SBUF bf16).
    - per sorted type-run piece, the table expansion is FUSED into the math
      as stride-0 broadcast operands (no expansion copies at all):
        a  = e * delta[bcast]; a2 = a + cb[bcast]   (DVE/GPSIMD greedy)
      th = Tanh(a2) per wave (ACT);  h = th * go[bcast] per piece.
    - h streams out bf16 (host upcasts).
  Emission order interleaves phase-1 chunks with the waves they unblock:
  engines execute queues in order, so phase 1 and phase 2 overlap.

Sharding: each core owns a 128-wide slice of H for all gate groups.
"""

import os

import numpy as np

HIDDEN = 1024
TYPES = 1001
TPAD = 1024
B = 32
T = 512
NTOK = B * T          # 16384
NCORES = 8
NGATES = 5            # i, z, o, ibar, d (f, fbar unused by the reference)
GATE_ROWS = (0, 2, 3, 4, 6)
KT = HIDDEN // 128    # 8 contraction tiles
TCHUNKS = ((0, 128), (128, 384), (512, 512))  # phase-1 type-chunk plan
SEGMAX = 1024         # max tokens per segment (PSUM tile for m)
WAVEMAX = 2048        # max tokens per wave (a2/th/h tiles)
OH_BOUNDS = (0, 2048, 8192, NTOK)             # oh input tile boundaries

LAST_RESULTS = None
_CACHED = None        # (ev_bytes_hash, nc)

# greedy piece-op lane costs (ns): fixed + per-column (HW-calibrated)
OP_COST = {"v": (380.0, 1.15), "g": (1100.0, 3.5)}


def _plan(ev_tok):
    """Host-side: count-sort types, sort tokens, and build the
    segment/wave/piece plan.

    Returns (type_order, new_id, perm, waves) where each wave is
    (w0, wlen, segs, pieces):
      seg   = (j0, slen, cc): tokens [j0, j0+slen) all have types in
              128-type chunk cc (types cc*128..cc*128+127 in NEW ids).
      piece = (t0, nt, reps, j0): tokens [j0, j0+nt*reps) are nt
              consecutive NEW types, each repeated reps times.
    """
    counts = np.bincount(ev_tok, minlength=TYPES)
    type_order = np.argsort(-counts, kind="stable")         # descending count
    new_id = np.empty(TYPES, np.int64)
    new_id[type_order] = np.arange(TYPES)
    key = new_id[ev_tok]
    perm = np.argsort(key, kind="stable")
    counts_sorted = counts[type_order]

    # raw per-type runs, cut at 128-type chunk boundaries into segments
    segs = []          # (j0, slen, cc)
    j = 0
    runs = []          # (t, j0, count)
    seg_j0, seg_cc = 0, 0
    for t in range(TYPES):
        c = int(counts_sorted[t])
        if c == 0:
            continue
        cc = t // 128
        if cc != seg_cc:
            if j > seg_j0:
                segs.append((seg_j0, j - seg_j0, seg_cc))
            seg_j0, seg_cc = j, cc
        runs.append((t, j, c))
        j += c
    if j > seg_j0:
        segs.append((seg_j0, j - seg_j0, seg_cc))
    assert j == NTOK
    # split long segments; also cut at oh-tile DMA boundaries
    segs2 = []
    for j0, slen, cc in segs:
        cuts = [b for b in OH_BOUNDS[1:-1] if j0 < b < j0 + slen]
        for b in cuts:
            segs2.append((j0, b - j0, cc))
            slen -= b - j0
            j0 = b
        segs2.append((j0, slen, cc))
    segs3 = []
    for j0, slen, cc in segs2:
        while slen > SEGMAX:
            segs3.append((j0, SEGMAX, cc))
            j0 += SEGMAX
            slen -= SEGMAX
        segs3.append((j0, slen, cc))
    segs2 = segs3
    # waves: pack consecutive segments up to WAVEMAX
    waves = []
    cur = []
    cur_len = 0
    for s in segs2:
        if cur and cur_len + s[1] > WAVEMAX:
            waves.append((cur[0][0], cur_len, cur, []))
            cur, cur_len = [], 0
        cur.append(s)
        cur_len += s[1]
    if cur:
        waves.append((cur[0][0], cur_len, cur, []))
    # pieces: cut runs at wave boundaries, merge equal-rep consecutive types
    # (only within one phase-1 type chunk, so pieces read a single tab tile)
    def tchunk_of(t):
        return 0 if t < 128 else (1 if t < 512 else 2)

    wbounds = [w[0] for w in waves] + [NTOK]
    wi = 0
    for t, j0, c in runs:
        left = c
        while left > 0:
            while j0 >= wbounds[wi + 1]:
                wi += 1
            take = min(left, wbounds[wi + 1] - j0)
            lst = waves[wi][3]
            if lst and lst[-1][2] == take and take == c \
                    and lst[-1][0] + lst[-1][1] == t \
                    and lst[-1][3] + lst[-1][1] * take == j0 \
                    and tchunk_of(lst[-1][0]) == tchunk_of(t):
                t0, nt, r, jj = lst[-1]
                lst[-1] = (t0, nt + 1, r, jj)
            else:
                lst.append((t, 1, take, j0))
            j0 += take
            left -= take
    return type_order, new_id, perm, waves


def _build_nc(waves):
    import concourse.mybir as mybir
    from concourse import bacc
    from concourse.tile import TileContext

    dt = mybir.dt
    AF = mybir.ActivationFunctionType
    f32 = dt.float32
    bf16 = dt.bfloat16

    nc = bacc.Bacc("TRN2", target_bir_lowering=False, debug=False)

    et_d = nc.dram_tensor("et", [HIDDEN, TPAD], bf16, kind="ExternalInput")
    wt_d = nc.dram_tensor("wt", [HIDDEN, NGATES * 128], bf16, kind="ExternalInput")
    bias_d = nc.dram_tensor("bias", [128, NGATES], f32, kind="ExternalInput")
    oh_d = nc.dram_tensor("oh", [128, NTOK], bf16, kind="ExternalInput")
    tab_d = nc.dram_tensor("tab", [128, 4, TPAD], bf16, kind="ExternalOutput")
    out_d = nc.dram_tensor("out", [128, NTOK], bf16, kind="ExternalOutput")

    with TileContext(nc) as tc:
        with (
            tc.tile_pool(name="const", bufs=1) as cpool,
            tc.tile_pool(name="p1ps", bufs=4, space="PSUM") as p1ps,
            tc.tile_pool(name="p2ps", bufs=2, space="PSUM") as p2ps,
            tc.tile_pool(name="p1t", bufs=2) as p1t,
            tc.tile_pool(name="wave", bufs=3) as wpool,
        ):
            # ---- constant loads. Every tile below is written by exactly one
            # DMA (or one phase-1 chunk): Tile dependency tracking is
            # per-tile, so multi-writer tiles serialize all their readers
            # behind the LAST writer.
            bias_sb = cpool.tile([128, NGATES], f32, tag="bias")
            nc.sync.dma_start(out=bias_sb[:], in_=bias_d[:])
            et_r = et_d[:].rearrange("(kt p) t -> p kt t", p=128)
            wt_r = wt_d[:].rearrange("(kt p) n -> p kt n", p=128)
            et_c = {}
            wt_g = {}
            oh_t = {}
            et_c[0] = cpool.tile([128, KT, 128], bf16, tag="et0", name="et_c0")
            nc.sync.dma_start(out=et_c[0][:], in_=et_r[:, :, 0:128])
            for g in (4, 0, 3, 2, 1):       # d-gate first: matmuls start sooner
                wt_g[g] = cpool.tile([128, KT, 128], bf16, tag=f"wt{g}", name=f"wt_g{g}")
                nc.sync.dma_start(
                    out=wt_g[g][:], in_=wt_r[:, :, g * 128:(g + 1) * 128])
            for bi in range(len(OH_BOUNDS) - 1):
                b0, b1 = OH_BOUNDS[bi], OH_BOUNDS[bi + 1]
                oh_t[bi] = cpool.tile([128, b1 - b0], bf16, tag=f"oh{bi}", name=f"oh_t{bi}")
                if bi == 0:
                    nc.sync.dma_start(out=oh_t[bi][:], in_=oh_d[:, b0:b1])
                elif bi == 1:
                    et_c[1] = cpool.tile([128, KT, 384], bf16, tag="et1", name="et_c1")
                    nc.sync.dma_start(out=et_c[1][:], in_=et_r[:, :, 128:512])
                    nc.sync.dma_start(out=oh_t[bi][:], in_=oh_d[:, b0:b1])
                else:
                    et_c[2] = cpool.tile([128, KT, 512], bf16, tag="et2", name="et_c2")
                    nc.sync.dma_start(out=et_c[2][:], in_=et_r[:, :, 512:1024])
                    nc.sync.dma_start(out=oh_t[bi][:], in_=oh_d[:, b0:b1])

            tab_c = {}
            gdT_c = {}

            def phase1_chunk(ci, c0, cn):
                tab = cpool.tile([128, 4, cn], bf16, tag=f"tab{ci}", name=f"tab{ci}")
                tab_c[ci] = tab
                # all activations from one ACT set (exp+tanh); sigmoids are
                # computed as sigmoid(x) = 0.5*tanh(x/2) + 0.5 (host sends
                # halved biases for gates i/o/ib). Only Ln needs a swap.
                tmp = {}
                tsp = None
                for g, func in (
                    (4, AF.Exp),            # d (softplus part 1)
                    (0, AF.Tanh),           # i (scaled)
                    (3, AF.Tanh),           # ibar (scaled)
                    (2, AF.Tanh),           # o (scaled)
                    (1, AF.Tanh),           # z
                ):
                    ps = p1ps.tile([128, 512], f32, tag="p1")
                    ps = ps[:, 0:cn]
                    for kt in range(KT):
                        nc.tensor.matmul(
                            ps, wt_g[g][:, kt, :],
                            et_c[ci][:, kt, :], start=kt == 0, stop=kt == KT - 1)
                    t = p1t.tile([128, 512], f32, tag=f"t{g}")
                    t = t[:, 0:cn]
                    scale = 0.5 if g in (0, 2, 3) else 1.0
                    nc.scalar.activation(
                        out=t, in_=ps, func=func,
                        bias=bias_sb[:, g:g + 1], scale=scale)
                    if g == 4:
                        tsp = t
                    else:
                        tmp[g] = t
                # softplus(d) = Ln(1 + Exp(d))  [the only table swap]
                nc.scalar.activation(
                    out=tab[:, 3, :], in_=tsp, func=AF.Ln, bias=1.0)
                # go = 0.5*t_o + 0.5
                nc.vector.tensor_scalar(
                    out=tab[:, 2, :], in0=tmp[2][:], scalar1=0.5, scalar2=0.5,
                    op0=mybir.AluOpType.mult, op1=mybir.AluOpType.add)
                # gz2 = 0.5*gz; delta = (t_i - t_ib)*gz2; cb = (t_ib + 1)*gz2
                gz2 = p1t.tile([128, 512], f32, tag="gz2")
                gz2 = gz2[:, 0:cn]
                nc.vector.tensor_scalar_mul(gz2, tmp[1][:], 0.5)
                d1 = p1t.tile([128, 512], f32, tag="d1")
                d1 = d1[:, 0:cn]
                nc.vector.tensor_sub(d1, tmp[0][:], tmp[3][:])
                nc.vector.tensor_mul(
                    out=tab[:, 0, :], in0=d1, in1=gz2)
                nc.vector.scalar_tensor_tensor(
                    out=tab[:, 1, :], in0=tmp[3][:], scalar=1.0, in1=gz2,
                    op0=mybir.AluOpType.add, op1=mybir.AluOpType.mult)
                # transpose gd rows of this chunk into matmul-stationary form
                for cc in range(c0 // 128, (c0 + cn) // 128):
                    gdT_c[cc] = cpool.tile([128, 128], bf16, tag=f"gdT{cc}", name=f"gdT{cc}")
                    lo = cc * 128 - c0
                    nc.sync.dma_start_transpose(
                        out=gdT_c[cc][:],
                        in_=tab[:, 3, lo:lo + 128])
                nc.sync.dma_start(
                    out=tab_d[:, :, c0:c0 + cn], in_=tab[:])

            def emit_wave(w0, wlen, segs, pieces):
                ws = slice(w0, w0 + wlen)
                e_t = wpool.tile([128, WAVEMAX], bf16, tag="e")
                a_t = wpool.tile([128, WAVEMAX], bf16, tag="a")
                a2_t = wpool.tile([128, WAVEMAX], bf16, tag="a2")
                th_t = wpool.tile([128, WAVEMAX], bf16, tag="th")
                # m = gd * (-dur) on PE; e = Exp(m) per segment
                for j0, slen, cc in segs:
                    bi = next(i for i in range(len(OH_BOUNDS) - 1)
                              if OH_BOUNDS[i] <= j0 and
                              j0 + slen <= OH_BOUNDS[i + 1])
                    jl = j0 - OH_BOUNDS[bi]
                    mps = p2ps.tile([128, SEGMAX], f32, tag="m")
                    mps = mps[:, 0:slen]
                    for q0 in range(0, slen, 512):
                        qn = min(512, slen - q0)
                        nc.tensor.matmul(
                            mps[:, q0:q0 + qn], gdT_c[cc][:],
                            oh_t[bi][:, jl + q0:jl + q0 + qn],
                            start=True, stop=True)
                    o0 = j0 - w0
                    nc.scalar.activation(
                        out=e_t[:, o0:o0 + slen], in_=mps, func=AF.Exp)
                # piece-level fused broadcast math, greedy v/g split
                load = {"v": 0.0, "g": 0.0}
                plan = []
                for (t0, nt, reps, j0) in sorted(pieces, key=lambda p: -p[1] * p[2]):
                    cols = nt * reps
                    cost = {k: f + cols * c for k, (f, c) in OP_COST.items()}
                    e = min(load, key=lambda k: load[k] + cost[k])
                    load[e] += 2 * cost[e]
                    plan.append((e, t0, nt, reps, j0))

                def bcast(slab, t0, nt, reps):
                    ci = 0 if t0 < 128 else (1 if t0 < 512 else 2)
                    tl = t0 - TCHUNKS[ci][0]
                    tab = tab_c[ci]
                    if nt == 1:
                        return tab[:, slab, tl:tl + 1].broadcast_to([128, reps])
                    return tab[:, slab, tl:tl + nt].unsqueeze(2).broadcast_to(
                        [128, nt, reps])

                for e, t0, nt, reps, j0 in plan:
                    o0 = j0 - w0
                    n = nt * reps
                    eng = nc.vector if e == "v" else nc.gpsimd
                    eng.tensor_mul(out=a_t[:, o0:o0 + n], in0=e_t[:, o0:o0 + n],
                                   in1=bcast(0, t0, nt, reps))
                    eng.tensor_add(out=a2_t[:, o0:o0 + n], in0=a_t[:, o0:o0 + n],
                                   in1=bcast(1, t0, nt, reps))
                nc.scalar.activation(
                    out=th_t[:, 0:wlen], in_=a2_t[:, 0:wlen], func=AF.Tanh)
                nc.sync.dma_start(out=out_d[:, ws], in_=th_t[:, 0:wlen])

            # ---- interleaved emission: each chunk, then the waves it unblocks
            wi = 0
            for ci, (c0, cn) in enumerate(TCHUNKS):
                phase1_chunk(ci, c0, cn)
                ready_types = c0 + cn if ci < len(TCHUNKS) - 1 else TPAD
                while wi < len(waves):
                    w0, wlen, segs, pieces = waves[wi]
                    maxtype = max(t0 + nt for t0, nt, _, _ in pieces)
                    if maxtype > ready_types:
                        break
                    emit_wave(w0, wlen, segs, pieces)
                    wi += 1
            assert wi == len(waves)

    nc.compile()
    return nc


def _marshal(event_seqs, duration_seqs, emb_table, W_rec, b_rec):
    import ml_dtypes

    ev = np.asarray(event_seqs)
    dur = np.asarray(duration_seqs, dtype=np.float32)
    emb = np.asarray(emb_table, dtype=np.float32)
    W = np.asarray(W_rec, dtype=np.float32)
    b = np.asarray(b_rec, dtype=np.float32)

    ev_tok = ev.T.reshape(-1)                      # token = t*B + b
    type_order, new_id, perm, waves = _plan(ev_tok)

    et = np.zeros((HIDDEN, TPAD), np.float32)
    et[:, :TYPES] = emb[type_order].T              # col t = NEW type id t
    et = et.astype(ml_dtypes.bfloat16)

    # scaled one-hot: row = NEW type id % 128, col = sorted token, val = -dur
    key_sorted = new_id[ev_tok][perm]
    oh = np.zeros((128, NTOK), np.float32)
    oh[key_sorted % 128, np.arange(NTOK)] = -dur.T.reshape(-1)[perm]
    oh = oh.astype(ml_dtypes.bfloat16)

    in_maps = []
    for k in range(NCORES):
        wt = np.empty((HIDDEN, NGATES * 128), np.float32)
        bias = np.empty((128, NGATES), np.float32)
        for g, g7 in enumerate(GATE_ROWS):
            rows = slice(g7 * HIDDEN + 128 * k, g7 * HIDDEN + 128 * (k + 1))
            wt[:, g * 128:(g + 1) * 128] = W[rows, :HIDDEN].T
            # sigmoid gates (i/o/ibar) run as tanh(x/2): halve their bias
            bias[:, g] = b[rows] * (0.5 if g in (0, 2, 3) else 1.0)
        in_maps.append({
            "et": et, "wt": wt.astype(ml_dtypes.bfloat16),
            "bias": bias, "oh": oh,
        })
    return ev_tok, new_id, perm, waves, in_maps


def _ensure_ntff_hook():
    import sys
    import types

    try:
        from antenv.axon_hooks import get_axon_ntff_profile_hook  # noqa: F401
        return
    except ImportError:
        pass
    try:
        import antenv
    except ImportError:
        return
    mod = types.ModuleType("antenv.axon_hooks")
    state = {"hook": None}
    mod.set_axon_ntff_profile_hook = lambda h: state.__setitem__("hook", h)
    mod.get_axon_ntff_profile_hook = lambda: state["hook"]
    sys.modules["antenv.axon_hooks"] = mod
    antenv.axon_hooks = mod
    try:
        from trn_agent_boot.trn_boot import _ntff_profile_via_ctypes

        hook = _ntff_profile_via_ctypes("/opt/axon/libaxon_pjrt.so")
        if hook is not None:
            mod.set_axon_ntff_profile_hook(hook)
    except Exception:
        pass


def kernel(event_seqs, duration_seqs, emb_table, W_rec, b_rec):
    global LAST_RESULTS, _CACHED
    from concourse.bass_utils import run_bass_kernel_spmd

    ev_tok, new_id, perm, waves, in_maps = _marshal(
        event_seqs, duration_seqs, emb_table, W_rec, b_rec)

    key = hash(np.asarray(event_seqs).tobytes())
    if _CACHED is None or _CACHED[0] != key:
        _CACHED = (key, _build_nc(waves))
    nc = _CACHED[1]

    trace = os.environ.get("KERNEL_TRACE", "") not in ("", "0")
    if trace:
        _ensure_ntff_hook()
    res = run_bass_kernel_spmd(nc, in_maps, list(range(NCORES)), trace=trace)
    LAST_RESULTS = res

    # ---- host-side output assembly ----------------------------------------
    # tables: [4 slabs, TPAD types, HIDDEN] f32, slab cols from each core
    tabT = np.empty((4, TPAD, HIDDEN), np.float32)
    for k in range(NCORES):
        tk = res.results[k]["tab"]                 # [128, 4, TPAD] bf16
        tabT[:, :, 128 * k:128 * (k + 1)] = \
            tk.astype(np.float32).transpose(1, 2, 0)
    key_raw = new_id[ev_tok]                       # per-token NEW type id

    full = np.empty((5, NTOK, HIDDEN), np.float32)
    np.add(tabT[0], tabT[1], out=tabT[0])          # c = delta + cb
    full[1] = tabT[0][key_raw]                     # c
    full[2] = tabT[1][key_raw]                     # c_bar
    full[3] = tabT[2][key_raw]                     # go
    full[4] = tabT[3][key_raw]                     # gd

    srt = np.empty((NTOK, HIDDEN), np.float32)
    for k in range(NCORES):
        hk = res.results[k]["out"]                 # th = tanh(c_d), [128, NTOK]
        srt[:, 128 * k:128 * (k + 1)] = hk.astype(np.float32).T
    full[0, perm, :] = srt
    np.multiply(full[0], full[3], out=full[0])     # h = go * tanh(c_d)
    return full.reshape(5, T, B, HIDDEN)


# revision 32
# speedup vs baseline: 1.0427x; 1.0427x over previous
"""CTLSTM Trainium2 kernel, v3b: per-type tables + h_d only, fused expansion.

The reference re-inits h/c/c_bar to zero every step, so all gate values
depend ONLY on the event type (1001 distinct embedding rows). Output planes
c, c_bar, go, gd are pure per-type gathers: the device returns the four
128x1024 bf16 tables and h_d only; the host's unshard step materializes
those planes with numpy fancy-indexing (same class of data movement as the
host un-permute it already does).

Device (per core, 128-wide H slice, all 16384 tokens host-sorted by type):
  phase 1: gate tables tab[128, 4(delta|cb|go|gd), 1024 types] via matmul
    (weights stationary, embedding columns moving) + ACT activations;
    delta = c - c_bar = (gi - gib)*gz tabulated. gd rows are DMA-transposed
    into gdT[128 types%128, chunk, 128 h] for use as matmul stationary.
  phase 2 (tokens sorted by type; segments are 128-type-chunk-aligned,
    waves pack segments):
    - m = gd * (-dur) comes off the TENSOR engine: one matmul per segment
      with stationary gdT[chunk] and moving a host-built scaled one-hot
      (row = type%128, value = -dur). No gd expansion, no DVE multiply.
    - e = Exp(m) per segment (ACT, PSUM->SBUF bf16).
    - per sorted type-run piece, the table expansion is FUSED into the math
      as stride-0 broadcast operands (no expansion copies at all):
        a  = e * delta[bcast]; a2 = a + cb[bcast]   (DVE/GPSIMD greedy)
      th = Tanh(a2) per wave (ACT);  h = th * go[bcast] per piece.
    - h streams out bf16 (host upcasts).
  Emission order interleaves phase-1 chunks with the waves they unblock:
  engines execute queues in order, so phase 1 and phase 2 overlap.

Sharding: each core owns a 128-wide slice of H for all gate groups.
"""

import os

import numpy as np

HIDDEN = 1024
TYPES = 1001
TPAD = 1024
B = 32
T = 512
NTOK = B * T          # 16384
NCORES = 8
NGATES = 5            # i, z, o, ibar, d (f, fbar unused by the reference)
GATE_ROWS = (0, 2, 3, 4, 6)
KT = HIDDEN // 128    # 8 contraction tiles
TCHUNKS = ((0, 128), (128, 384), (512, 512))  # phase-1 type-chunk plan
SEGMAX = 512          # max tokens per segment (PSUM tile for m)
WAVEMAX = 2048        # max tokens per wave (a2/th/h tiles)
OH_BOUNDS = (0, 2048, 8192, NTOK)             # oh input tile boundaries

LAST_RESULTS = None
_CACHED = None        # (ev_bytes_hash, nc)

# greedy piece-op lane costs (ns): fixed + per-column (HW-calibrated)
OP_COST = {"v": (380.0, 1.15), "g": (1100.0, 3.5)}


def _plan(ev_tok):
    """Host-side: count-sort types, sort tokens, and build the
    segment/wave/piece plan.

    Returns (type_order, new_id, perm, waves) where each wave is
    (w0, wlen, segs, pieces):
      seg   = (j0, slen, cc): tokens [j0, j0+slen) all have types in
              128-type chunk cc (types cc*128..cc*128+127 in NEW ids).
      piece = (t0, nt, reps, j0): tokens [j0, j0+nt*reps) are nt
              consecutive NEW types, each repeated reps times.
    """
    counts = np.bincount(ev_tok, minlength=TYPES)
    type_order = np.argsort(-counts, kind="stable")         # descending count
    new_id = np.empty(TYPES, np.int64)
    new_id[type_order] = np.arange(TYPES)
    key = new_id[ev_tok]
    perm = np.argsort(key, kind="stable")
    counts_sorted = counts[type_order]

    # raw per-type runs, cut at 128-type chunk boundaries into segments
    segs = []          # (j0, slen, cc)
    j = 0
    runs = []          # (t, j0, count)
    seg_j0, seg_cc = 0, 0
    for t in range(TYPES):
        c = int(counts_sorted[t])
        if c == 0:
            continue
        cc = t // 128
        if cc != seg_cc:
            if j > seg_j0:
                segs.append((seg_j0, j - seg_j0, seg_cc))
            seg_j0, seg_cc = j, cc
        runs.append((t, j, c))
        j += c
    if j > seg_j0:
        segs.append((seg_j0, j - seg_j0, seg_cc))
    assert j == NTOK
    # split long segments; also cut at oh-tile DMA boundaries
    segs2 = []
    for j0, slen, cc in segs:
        cuts = [b for b in OH_BOUNDS[1:-1] if j0 < b < j0 + slen]
        for b in cuts:
            segs2.append((j0, b - j0, cc))
            slen -= b - j0
            j0 = b
        segs2.append((j0, slen, cc))
    segs3 = []
    for j0, slen, cc in segs2:
        while slen > SEGMAX:
            segs3.append((j0, SEGMAX, cc))
            j0 += SEGMAX
            slen -= SEGMAX
        segs3.append((j0, slen, cc))
    segs2 = segs3
    # waves: pack consecutive segments up to WAVEMAX
    waves = []
    cur = []
    cur_len = 0
    for s in segs2:
        if cur and cur_len + s[1] > WAVEMAX:
            waves.append((cur[0][0], cur_len, cur, []))
            cur, cur_len = [], 0
        cur.append(s)
        cur_len += s[1]
    if cur:
        waves.append((cur[0][0], cur_len, cur, []))
    # pieces: cut runs at wave boundaries, merge equal-rep consecutive types
    # (only within one phase-1 type chunk, so pieces read a single tab tile)
    def tchunk_of(t):
        return 0 if t < 128 else (1 if t < 512 else 2)

    wbounds = [w[0] for w in waves] + [NTOK]
    wi = 0
    for t, j0, c in runs:
        left = c
        while left > 0:
            while j0 >= wbounds[wi + 1]:
                wi += 1
            take = min(left, wbounds[wi + 1] - j0)
            lst = waves[wi][3]
            if lst and lst[-1][2] == take and take == c \
                    and lst[-1][0] + lst[-1][1] == t \
                    and lst[-1][3] + lst[-1][1] * take == j0 \
                    and tchunk_of(lst[-1][0]) == tchunk_of(t):
                t0, nt, r, jj = lst[-1]
                lst[-1] = (t0, nt + 1, r, jj)
            else:
                lst.append((t, 1, take, j0))
            j0 += take
            left -= take
    return type_order, new_id, perm, waves


def _build_nc(waves):
    import concourse.mybir as mybir
    from concourse import bacc
    from concourse.masks import make_identity
    from concourse.tile import TileContext

    dt = mybir.dt
    AF = mybir.ActivationFunctionType
    f32 = dt.float32
    bf16 = dt.bfloat16

    nc = bacc.Bacc("TRN2", target_bir_lowering=False, debug=False)

    et_d = nc.dram_tensor("et", [HIDDEN, TPAD], bf16, kind="ExternalInput")
    wt_d = nc.dram_tensor("wt", [HIDDEN, NGATES * 128], bf16, kind="ExternalInput")
    bias_d = nc.dram_tensor("bias", [128, NGATES], f32, kind="ExternalInput")
    oh_d = nc.dram_tensor("oh", [128, NTOK], bf16, kind="ExternalInput")
    tab_d = nc.dram_tensor("tab", [128, 4, TPAD], bf16, kind="ExternalOutput")
    out_d = nc.dram_tensor("out", [128, NTOK], bf16, kind="ExternalOutput")

    with TileContext(nc) as tc:
        with (
            tc.tile_pool(name="const", bufs=1) as cpool,
            tc.tile_pool(name="p1ps", bufs=3, space="PSUM") as p1ps,
            tc.tile_pool(name="p2ps", bufs=3, space="PSUM") as p2ps,
            tc.tile_pool(name="tpps", bufs=1, space="PSUM") as tpps,
            tc.tile_pool(name="p1t", bufs=2) as p1t,
            tc.tile_pool(name="wave", bufs=3) as wpool,
        ):
            # ---- constant loads. Every tile below is written by exactly one
            # DMA (or one phase-1 chunk): Tile dependency tracking is
            # per-tile, so multi-writer tiles serialize all their readers
            # behind the LAST writer.
            bias_sb = cpool.tile([128, NGATES], f32, tag="bias")
            nc.sync.dma_start(out=bias_sb[:], in_=bias_d[:])
            ident = cpool.tile([128, 128], bf16, tag="ident")
            make_identity(nc, ident[:])
            et_r = et_d[:].rearrange("(kt p) t -> p kt t", p=128)
            wt_r = wt_d[:].rearrange("(kt p) n -> p kt n", p=128)
            et_c = {}
            wt_g = {}
            oh_t = {}
            et_c[0] = cpool.tile([128, KT, 128], bf16, tag="et0", name="et_c0")
            nc.sync.dma_start(out=et_c[0][:], in_=et_r[:, :, 0:128])
            for g in (4, 0, 3, 2, 1):       # d-gate first: matmuls start sooner
                wt_g[g] = cpool.tile([128, KT, 128], bf16, tag=f"wt{g}", name=f"wt_g{g}")
                nc.sync.dma_start(
                    out=wt_g[g][:], in_=wt_r[:, :, g * 128:(g + 1) * 128])
            for bi in range(len(OH_BOUNDS) - 1):
                b0, b1 = OH_BOUNDS[bi], OH_BOUNDS[bi + 1]
                oh_t[bi] = cpool.tile([128, b1 - b0], bf16, tag=f"oh{bi}",
                                      name=f"oh_t{bi}")
            nc.sync.dma_start(out=oh_t[0][:], in_=oh_d[:, 0:OH_BOUNDS[1]])
            et_c[1] = cpool.tile([128, KT, 384], bf16, tag="et1", name="et_c1")
            nc.sync.dma_start(out=et_c[1][:], in_=et_r[:, :, 128:512])
            et_c[2] = cpool.tile([128, KT, 512], bf16, tag="et2", name="et_c2")
            nc.sync.dma_start(out=et_c[2][:], in_=et_r[:, :, 512:1024])

            tab_c = {}
            gdT_c = {}

            def phase1_chunk(ci, c0, cn):
                tab = cpool.tile([128, 4, cn], bf16, tag=f"tab{ci}", name=f"tab{ci}")
                tab_c[ci] = tab
                # all activations from one ACT set (exp+tanh); sigmoids are
                # computed as sigmoid(x) = 0.5*tanh(x/2) + 0.5 (host sends
                # halved biases for gates i/o/ib). Only Ln needs a swap.
                tmp = {}
                tsp = None
                for g, func in (
                    (4, AF.Exp),            # d (softplus part 1)
                    (0, AF.Tanh),           # i (scaled)
                    (3, AF.Tanh),           # ibar (scaled)
                    (2, AF.Tanh),           # o (scaled)
                    (1, AF.Tanh),           # z
                ):
                    ps = p1ps.tile([128, 512], f32, tag="p1")
                    ps = ps[:, 0:cn]
                    for kt in range(KT):
                        nc.tensor.matmul(
                            ps, wt_g[g][:, kt, :],
                            et_c[ci][:, kt, :], start=kt == 0, stop=kt == KT - 1)
                    t = p1t.tile([128, 512], f32, tag=f"t{g}")
                    t = t[:, 0:cn]
                    scale = 0.5 if g in (0, 2, 3) else 1.0
                    nc.scalar.activation(
                        out=t, in_=ps, func=func,
                        bias=bias_sb[:, g:g + 1], scale=scale)
                    if g == 4:
                        tsp = t
                    else:
                        tmp[g] = t
                # softplus(d) = Ln(1 + Exp(d))  [the only table swap]
                nc.scalar.activation(
                    out=tab[:, 3, :], in_=tsp, func=AF.Ln, bias=1.0)
                # go = 0.5*t_o + 0.5
                nc.vector.tensor_scalar(
                    out=tab[:, 2, :], in0=tmp[2][:], scalar1=0.5, scalar2=0.5,
                    op0=mybir.AluOpType.mult, op1=mybir.AluOpType.add)
                # gz2 = 0.5*gz; delta = (t_i - t_ib)*gz2; cb = (t_ib + 1)*gz2
                gz2 = p1t.tile([128, 512], f32, tag="gz2")
                gz2 = gz2[:, 0:cn]
                nc.vector.tensor_scalar_mul(gz2, tmp[1][:], 0.5)
                d1 = p1t.tile([128, 512], f32, tag="d1")
                d1 = d1[:, 0:cn]
                nc.vector.tensor_sub(d1, tmp[0][:], tmp[3][:])
                nc.vector.tensor_mul(
                    out=tab[:, 0, :], in0=d1, in1=gz2)
                nc.vector.scalar_tensor_tensor(
                    out=tab[:, 1, :], in0=tmp[3][:], scalar=1.0, in1=gz2,
                    op0=mybir.AluOpType.add, op1=mybir.AluOpType.mult)
                # transpose gd rows of this chunk into matmul-stationary form
                # (PE transpose: no dependence on congested DMA queues)
                for cc in range(c0 // 128, (c0 + cn) // 128):
                    gdT_c[cc] = cpool.tile([128, 128], bf16, tag=f"gdT{cc}",
                                           name=f"gdT{cc}")
                    lo = cc * 128 - c0
                    tps = tpps.tile([128, 128], bf16, tag="tp")
                    nc.tensor.transpose(tps[:], tab[:, 3, lo:lo + 128], ident[:])
                    nc.vector.tensor_copy(out=gdT_c[cc][:], in_=tps[:])

            def emit_wave(w0, wlen, segs, pieces):
                ws = slice(w0, w0 + wlen)
                e_t = wpool.tile([128, WAVEMAX], bf16, tag="e")
                a_t = wpool.tile([128, WAVEMAX], bf16, tag="a")
                a2_t = wpool.tile([128, WAVEMAX], bf16, tag="a2")
                th_t = wpool.tile([128, WAVEMAX], bf16, tag="th")
                # m = gd * (-dur) on PE; e = Exp(m) per segment
                for j0, slen, cc in segs:
                    bi = next(i for i in range(len(OH_BOUNDS) - 1)
                              if OH_BOUNDS[i] <= j0 and
                              j0 + slen <= OH_BOUNDS[i + 1])
                    jl = j0 - OH_BOUNDS[bi]
                    mps = p2ps.tile([128, SEGMAX], f32, tag="m")
                    mps = mps[:, 0:slen]
                    for q0 in range(0, slen, 512):
                        qn = min(512, slen - q0)
                        nc.tensor.matmul(
                            mps[:, q0:q0 + qn], gdT_c[cc][:],
                            oh_t[bi][:, jl + q0:jl + q0 + qn],
                            start=True, stop=True)
                    o0 = j0 - w0
                    nc.scalar.activation(
                        out=e_t[:, o0:o0 + slen], in_=mps, func=AF.Exp)
                # piece-level fused broadcast math, greedy v/g split
                load = {"v": 0.0, "g": 0.0}
                plan = []
                for (t0, nt, reps, j0) in sorted(pieces, key=lambda p: -p[1] * p[2]):
                    cols = nt * reps
                    cost = {k: f + cols * c for k, (f, c) in OP_COST.items()}
                    e = min(load, key=lambda k: load[k] + cost[k])
                    load[e] += 2 * cost[e]
                    plan.append((e, t0, nt, reps, j0))

                def bcast(slab, t0, nt, reps):
                    ci = 0 if t0 < 128 else (1 if t0 < 512 else 2)
                    tl = t0 - TCHUNKS[ci][0]
                    tab = tab_c[ci]
                    if nt == 1:
                        return tab[:, slab, tl:tl + 1].broadcast_to([128, reps])
                    return tab[:, slab, tl:tl + nt].unsqueeze(2).broadcast_to(
                        [128, nt, reps])

                for e, t0, nt, reps, j0 in plan:
                    o0 = j0 - w0
                    n = nt * reps
                    eng = nc.vector if e == "v" else nc.gpsimd
                    eng.tensor_mul(out=a_t[:, o0:o0 + n], in0=e_t[:, o0:o0 + n],
                                   in1=bcast(0, t0, nt, reps))
                    eng.tensor_add(out=a2_t[:, o0:o0 + n], in0=a_t[:, o0:o0 + n],
                                   in1=bcast(1, t0, nt, reps))
                nc.scalar.activation(
                    out=th_t[:, 0:wlen], in_=a2_t[:, 0:wlen], func=AF.Tanh)
                nc.sync.dma_start(out=out_d[:, ws], in_=th_t[:, 0:wlen])

            # ---- interleaved emission: each chunk, then the waves it unblocks
            wi = 0
            for ci, (c0, cn) in enumerate(TCHUNKS):
                phase1_chunk(ci, c0, cn)
                if ci < len(OH_BOUNDS) - 2:      # big oh pieces load late
                    b0, b1 = OH_BOUNDS[ci + 1], OH_BOUNDS[ci + 2]
                    nc.sync.dma_start(out=oh_t[ci + 1][:], in_=oh_d[:, b0:b1])
                ready_types = c0 + cn if ci < len(TCHUNKS) - 1 else TPAD
                while wi < len(waves):
                    w0, wlen, segs, pieces = waves[wi]
                    maxtype = max(t0 + nt for t0, nt, _, _ in pieces)
                    if maxtype > ready_types:
                        break
                    emit_wave(w0, wlen, segs, pieces)
                    wi += 1
            assert wi == len(waves)
            for ci, (c0, cn) in enumerate(TCHUNKS):
                nc.sync.dma_start(
                    out=tab_d[:, :, c0:c0 + cn], in_=tab_c[ci][:])

    nc.compile()
    return nc


def _marshal(event_seqs, duration_seqs, emb_table, W_rec, b_rec):
    import ml_dtypes

    ev = np.asarray(event_seqs)
    dur = np.asarray(duration_seqs, dtype=np.float32)
    emb = np.asarray(emb_table, dtype=np.float32)
    W = np.asarray(W_rec, dtype=np.float32)
    b = np.asarray(b_rec, dtype=np.float32)

    ev_tok = ev.T.reshape(-1)                      # token = t*B + b
    type_order, new_id, perm, waves = _plan(ev_tok)

    et = np.zeros((HIDDEN, TPAD), np.float32)
    et[:, :TYPES] = emb[type_order].T              # col t = NEW type id t
    et = et.astype(ml_dtypes.bfloat16)

    # scaled one-hot: row = NEW type id % 128, col = sorted token, val = -dur
    key_sorted = new_id[ev_tok][perm]
    oh = np.zeros((128, NTOK), np.float32)
    oh[key_sorted % 128, np.arange(NTOK)] = -dur.T.reshape(-1)[perm]
    oh = oh.astype(ml_dtypes.bfloat16)

    in_maps = []
    for k in range(NCORES):
        wt = np.empty((HIDDEN, NGATES * 128), np.float32)
        bias = np.empty((128, NGATES), np.float32)
        for g, g7 in enumerate(GATE_ROWS):
            rows = slice(g7 * HIDDEN + 128 * k, g7 * HIDDEN + 128 * (k + 1))
            wt[:, g * 128:(g + 1) * 128] = W[rows, :HIDDEN].T
            # sigmoid gates (i/o/ibar) run as tanh(x/2): halve their bias
            bias[:, g] = b[rows] * (0.5 if g in (0, 2, 3) else 1.0)
        in_maps.append({
            "et": et, "wt": wt.astype(ml_dtypes.bfloat16),
            "bias": bias, "oh": oh,
        })
    return ev_tok, new_id, perm, waves, in_maps


def _ensure_ntff_hook():
    import sys
    import types

    try:
        from antenv.axon_hooks import get_axon_ntff_profile_hook  # noqa: F401
        return
    except ImportError:
        pass
    try:
        import antenv
    except ImportError:
        return
    mod = types.ModuleType("antenv.axon_hooks")
    state = {"hook": None}
    mod.set_axon_ntff_profile_hook = lambda h: state.__setitem__("hook", h)
    mod.get_axon_ntff_profile_hook = lambda: state["hook"]
    sys.modules["antenv.axon_hooks"] = mod
    antenv.axon_hooks = mod
    try:
        from trn_agent_boot.trn_boot import _ntff_profile_via_ctypes

        hook = _ntff_profile_via_ctypes("/opt/axon/libaxon_pjrt.so")
        if hook is not None:
            mod.set_axon_ntff_profile_hook(hook)
    except Exception:
        pass


def kernel(event_seqs, duration_seqs, emb_table, W_rec, b_rec):
    global LAST_RESULTS, _CACHED
    from concourse.bass_utils import run_bass_kernel_spmd

    ev_tok, new_id, perm, waves, in_maps = _marshal(
        event_seqs, duration_seqs, emb_table, W_rec, b_rec)

    key = hash(np.asarray(event_seqs).tobytes())
    if _CACHED is None or _CACHED[0] != key:
        _CACHED = (key, _build_nc(waves))
    nc = _CACHED[1]

    trace = os.environ.get("KERNEL_TRACE", "") not in ("", "0")
    if trace:
        _ensure_ntff_hook()
    res = run_bass_kernel_spmd(nc, in_maps, list(range(NCORES)), trace=trace)
    LAST_RESULTS = res

    # ---- host-side output assembly ----------------------------------------
    # tables: [4 slabs, TPAD types, HIDDEN] f32, slab cols from each core
    tabT = np.empty((4, TPAD, HIDDEN), np.float32)
    for k in range(NCORES):
        tk = res.results[k]["tab"]                 # [128, 4, TPAD] bf16
        tabT[:, :, 128 * k:128 * (k + 1)] = \
            tk.astype(np.float32).transpose(1, 2, 0)
    key_raw = new_id[ev_tok]                       # per-token NEW type id

    full = np.empty((5, NTOK, HIDDEN), np.float32)
    np.add(tabT[0], tabT[1], out=tabT[0])          # c = delta + cb
    full[1] = tabT[0][key_raw]                     # c
    full[2] = tabT[1][key_raw]                     # c_bar
    full[3] = tabT[2][key_raw]                     # go
    full[4] = tabT[3][key_raw]                     # gd

    srt = np.empty((NTOK, HIDDEN), np.float32)
    for k in range(NCORES):
        hk = res.results[k]["out"]                 # th = tanh(c_d), [128, NTOK]
        srt[:, 128 * k:128 * (k + 1)] = hk.astype(np.float32).T
    full[0, perm, :] = srt
    np.multiply(full[0], full[3], out=full[0])     # h = go * tanh(c_d)
    return full.reshape(5, T, B, HIDDEN)
